# revision 23
# baseline (speedup 1.0000x reference)
"""Trainium2 Bass kernel for nn_CausalSelfAttention_52012053954857.

Full-input contract: kernel(**inputs) takes the unsharded fp32 inputs,
shards across 8 NeuronCores (tensor-parallel over the 4 KV-head groups x
data-parallel over batch 2), runs one SPMD Bass/Tile kernel, and gathers
(sum over TP ranks, stack over batch) the full [2, 2048, 2048] output.

Per-core dataflow (all matmuls bf16 with fp32 PSUM accumulation):
  x-tiles (stationary) @ wq/wk/wv -> q,k,v in [tokens, dims] layout
  RMSNorm via ACT square+rsqrt (norm weights + 1/sqrt(HD) folded into
  host-precomputed rope tables), RoPE on DVE, cast bf16
  PE-transpose q,k into [HD, T]; v stays [T, HD]
  scores^T[kpos, qpos] = kT_block.T @ qT_chunk (causal: restricted N)
  diagonal-block causal mask added in PSUM, exp on ACT -> probs bf16
  y^T[HD, qpos] accumulates v.T @ probs^T in PSUM over k-blocks
  softmax denominator: DVE-accumulated probs, ones-matmul partition sum,
  ACT reciprocal, gpsimd partition_broadcast, DVE multiply -> y bf16
  out rows = y^T.T @ woT, summed over the 4 heads, fp32 out.
"""
import os
import sys

for _p in ("/opt/trn_rl_repo", "/opt/pypackages"):
    if _p not in sys.path and os.path.isdir(_p):
        sys.path.append(_p)

import numpy as np
import ml_dtypes

import concourse.mybir as mybir
import concourse.tile as tile
from concourse import bacc
from concourse.bass_utils import run_bass_kernel_spmd

BF16 = ml_dtypes.bfloat16

B, T, C = 2, 2048, 2048
N_HEAD, N_KV, HD = 16, 4, 128
NR = N_HEAD // N_KV          # 4 q-heads per kv group
G_HD = NR * HD               # 512 q dims per group
THETA = 500000.0
EPS = 1e-6
P = 128
TB = T // P                  # 16 token blocks
KC = C // P                  # 16 contraction chunks
QCH = 512                    # q-chunk width for attention
NQC = T // QCH               # 4 q chunks
QB = QCH // P                # 4 token blocks per q chunk
MASK_NEG = -1e9

_CACHED = {}


def _build_nc():
    nc = bacc.Bacc("TRN2", target_bir_lowering=False, debug=False, num_devices=8)
    f32, bf16 = mybir.dt.float32, mybir.dt.bfloat16

    xt_d = nc.dram_tensor("xt", [TB, P, KC, P], bf16, kind="ExternalInput")
    wq_d = nc.dram_tensor("wqT", [P, KC, G_HD], bf16, kind="ExternalInput")
    wk_d = nc.dram_tensor("wkT", [P, KC, HD], bf16, kind="ExternalInput")
    wv_d = nc.dram_tensor("wvT", [P, KC, HD], bf16, kind="ExternalInput")
    wo_d = nc.dram_tensor("woT", [P, NR, C], bf16, kind="ExternalInput")
    cq_d = nc.dram_tensor("cosq", [P, TB, HD], f32, kind="ExternalInput")
    sq_d = nc.dram_tensor("sinq", [P, TB, HD], f32, kind="ExternalInput")
    ck_d = nc.dram_tensor("cosk", [P, TB, HD], f32, kind="ExternalInput")
    sk_d = nc.dram_tensor("sink", [P, TB, HD], f32, kind="ExternalInput")
    mask_d = nc.dram_tensor("mask", [P, P], f32, kind="ExternalInput")
    ident_d = nc.dram_tensor("ident", [P, P], bf16, kind="ExternalInput")
    ones_d = nc.dram_tensor("ones", [P, 1], bf16, kind="ExternalInput")
    out_d = nc.dram_tensor("out", [T, C], f32, kind="ExternalOutput")

    with tile.TileContext(nc) as tc:
        with (
            tc.tile_pool(name="const", bufs=1) as const,
            tc.tile_pool(name="xp", bufs=3) as xp,
            tc.tile_pool(name="work", bufs=2) as work,
            tc.tile_pool(name="probs", bufs=6) as ppool,
            tc.tile_pool(name="ost", bufs=2) as ost,
            tc.tile_pool(name="psA", bufs=3, space="PSUM") as psA,   # q-proj / scores
            tc.tile_pool(name="psB", bufs=2, space="PSUM") as psB,   # kv-proj / y
            tc.tile_pool(name="psC", bufs=2, space="PSUM") as psC,   # ds / out-proj
        ):
            # ---- persistent SBUF ----
            wq_sb = const.tile([P, KC, G_HD], bf16, tag="wq")
            wk_sb = const.tile([P, KC, HD], bf16, tag="wk")
            wv_sb = const.tile([P, KC, HD], bf16, tag="wv")
            wo_sb = const.tile([P, NR, C], bf16, tag="wo")
            cq_sb = const.tile([P, TB, HD], f32, tag="cq")
            sq_sb = const.tile([P, TB, HD], f32, tag="sq")
            ck_sb = const.tile([P, TB, HD], f32, tag="ck")
            sk_sb = const.tile([P, TB, HD], f32, tag="sk")
            mask_sb = const.tile([P, P], f32, tag="mask")
            ident_sb = const.tile([P, P], bf16, tag="ident")
            ones_sb = const.tile([P, 1], bf16, tag="ones")
            qT_sb = [const.tile([P, T], bf16, tag=f"qT{h}", name=f"qT{h}")
                     for h in range(NR)]
            kT_sb = const.tile([P, T], bf16, tag="kT")
            v_sb = const.tile([P, TB, HD], bf16, tag="v")

            nc.sync.dma_start(wq_sb[:], wq_d[:])
            nc.sync.dma_start(wk_sb[:], wk_d[:])
            nc.sync.dma_start(wv_sb[:], wv_d[:])
            nc.sync.dma_start(wo_sb[:], wo_d[:])
            nc.sync.dma_start(cq_sb[:], cq_d[:])
            nc.sync.dma_start(sq_sb[:], sq_d[:])
            nc.sync.dma_start(ck_sb[:], ck_d[:])
            nc.sync.dma_start(sk_sb[:], sk_d[:])
            nc.sync.dma_start(mask_sb[:], mask_d[:])
            nc.sync.dma_start(ident_sb[:], ident_d[:])
            nc.sync.dma_start(ones_sb[:], ones_d[:])

            epsq_sb = const.tile([P, 1], f32, tag="epsq")
            epsk_sb = const.tile([P, 1], f32, tag="epsk")
            nc.gpsimd.memset(epsq_sb[:], float(HD * EPS))
            nc.gpsimd.memset(epsk_sb[:], float(EPS))

            ybf = [None] * NR  # per-head normalized y^T [HD, QCH] bf16, per q-chunk
            proj_state = {}    # tb -> (q_ps, kv_ps)

            def proj_front(tb):
                """QKV projection matmuls for token block tb."""
                xt = xp.tile([P, KC, P], bf16, tag="xt")
                nc.sync.dma_start(xt[:], xt_d[tb])

                q_ps = psA.tile([P, G_HD], f32, tag="A")
                kv_ps = psB.tile([P, 2, HD], f32, tag="B")
                for kc in range(KC):
                    nc.tensor.matmul(q_ps[:], xt[:, kc], wq_sb[:, kc],
                                     start=(kc == 0), stop=(kc == KC - 1))
                for kc in range(KC):
                    nc.tensor.matmul(kv_ps[:, 0], xt[:, kc], wk_sb[:, kc],
                                     start=(kc == 0), stop=(kc == KC - 1))
                for kc in range(KC):
                    nc.tensor.matmul(kv_ps[:, 1], xt[:, kc], wv_sb[:, kc],
                                     start=(kc == 0), stop=(kc == KC - 1))
                proj_state[tb] = (q_ps, kv_ps)

            def proj_tail(tb):
                """Norm + rope + transposes for token block tb."""
                q_ps, kv_ps = proj_state.pop(tb)

                # v: straight cast copy into persistent [kpos, HD] store
                nc.scalar.activation(v_sb[:, tb], kv_ps[:, 1],
                                     mybir.ActivationFunctionType.Copy)

                # ---- RMSNorm stats (ACT square -> DVE segmented reduce) ----
                sq_scr = work.tile([P, NR + 1, HD], f32, tag="sq_scr")
                sums = work.tile([P, NR + 1], f32, tag="sums")
                rinv = work.tile([P, NR + 1], f32, tag="rinv")
                q3 = q_ps[:].rearrange("p (r d) -> p r d", r=NR)
                sq_flat = sq_scr[:].rearrange("p r d -> p (r d)")
                nc.scalar.activation(sq_flat[:, 0:G_HD], q_ps[:],
                                     mybir.ActivationFunctionType.Square)
                nc.scalar.activation(sq_flat[:, G_HD:G_HD + HD], kv_ps[:, 0],
                                     mybir.ActivationFunctionType.Square)
                nc.vector.reduce_sum(sums[:], sq_scr[:], axis=mybir.AxisListType.X)
                # q: rsqrt(sum + HD*eps) = rsqrt(mean+eps)/sqrt(HD)  (scores scale folded)
                # k: rsqrt(mean + eps); ACT Rsqrt is banned -> Sqrt + DVE reciprocal
                rt = work.tile([P, NR + 1], f32, tag="rt")
                nc.scalar.activation(rt[:, 0:NR], sums[:, 0:NR],
                                     mybir.ActivationFunctionType.Sqrt,
                                     bias=epsq_sb[:], scale=1.0)
                nc.scalar.activation(rt[:, NR:NR + 1], sums[:, NR:NR + 1],
                                     mybir.ActivationFunctionType.Sqrt,
                                     bias=epsk_sb[:], scale=1.0 / HD)
                nc.vector.reciprocal_approx_fast(rinv[:], rt[:])

                # ---- RoPE (DVE) ----
                # q
                rot = work.tile([P, NR, 2, 64], f32, tag="rot")
                q4 = q_ps[:].rearrange("p (r s d) -> p r s d", r=NR, s=2)
                nc.vector.tensor_scalar_mul(rot[:, :, 0, :], q4[:, :, 1, :], -1.0)
                nc.vector.tensor_copy(rot[:, :, 1, :], q4[:, :, 0, :])
                rot3 = rot[:].rearrange("p r s d -> p r (s d)")
                t1 = work.tile([P, NR, HD], f32, tag="t1")
                t2 = work.tile([P, NR, HD], f32, tag="t2")
                cq_b = cq_sb[:, tb:tb + 1, :].to_broadcast((P, NR, HD))
                sq_b = sq_sb[:, tb:tb + 1, :].to_broadcast((P, NR, HD))
                nc.vector.tensor_tensor(t1[:], q3, cq_b, mybir.AluOpType.mult)
                nc.vector.tensor_tensor(t2[:], rot3, sq_b, mybir.AluOpType.mult)
                qr = work.tile([P, NR, HD], f32, tag="qr")
                nc.vector.tensor_tensor(qr[:], t1[:], t2[:], mybir.AluOpType.add)
                qbf = work.tile([P, NR, HD], bf16, tag="qbf")
                rinvq_b = rinv[:, 0:NR, None].to_broadcast((P, NR, HD))
                nc.vector.tensor_tensor(qbf[:], qr[:], rinvq_b, mybir.AluOpType.mult)
                # k
                krot = work.tile([P, 2, 64], f32, tag="krot")
                k3 = kv_ps[:, 0].rearrange("p (s d) -> p s d", s=2)
                nc.vector.tensor_scalar_mul(krot[:, 0, :], k3[:, 1, :], -1.0)
                nc.vector.tensor_copy(krot[:, 1, :], k3[:, 0, :])
                krot2 = krot[:].rearrange("p s d -> p (s d)")
                kt1 = work.tile([P, HD], f32, tag="kt1")
                kt2 = work.tile([P, HD], f32, tag="kt2")
                nc.vector.tensor_tensor(kt1[:], kv_ps[:, 0], ck_sb[:, tb], mybir.AluOpType.mult)
                nc.vector.tensor_tensor(kt2[:], krot2, sk_sb[:, tb], mybir.AluOpType.mult)
                kr = work.tile([P, HD], f32, tag="kr")
                nc.vector.tensor_tensor(kr[:], kt1[:], kt2[:], mybir.AluOpType.add)
                kbf = work.tile([P, HD], bf16, tag="kbf")
                rinvk_b = rinv[:, NR:NR + 1].to_broadcast((P, HD))
                nc.vector.tensor_tensor(kbf[:], kr[:], rinvk_b, mybir.AluOpType.mult)

                # ---- transposes to [HD, T] (PE) ----
                qbf2 = qbf[:].rearrange("p r d -> p (r d)")
                for h in range(NR):
                    tp = psC.tile([P, P], bf16, tag="C")
                    nc.tensor.transpose(tp[:], qbf2[:, h * HD:(h + 1) * HD], ident_sb[:])
                    nc.vector.tensor_copy(qT_sb[h][:, tb * P:(tb + 1) * P], tp[:])
                tp = psC.tile([P, P], bf16, tag="C")
                nc.tensor.transpose(tp[:], kbf[:], ident_sb[:])
                nc.vector.tensor_copy(kT_sb[:, tb * P:(tb + 1) * P], tp[:])

            attn_state = {}  # h -> (y_ps, acc0, acc1)
            LOOKAHEAD = 2

            def attn_front(h, qc):
                """QK + exp + AV for head h, q-chunk qc (AV trails QK by
                LOOKAHEAD so exp latency hides under PE work)."""
                nkb = QB * qc + QB
                y_ps = psB.tile([P, QCH], f32, tag="B")
                ds_ps = psC.tile([1, QCH], f32, tag="C")
                pbs = [None] * nkb
                offs = [0] * nkb

                def emit_av(kb):
                    nn = QCH - offs[kb]
                    nc.tensor.matmul(y_ps[:, offs[kb]:], v_sb[:, kb],
                                     pbs[kb][:, :nn],
                                     start=(kb == 0), stop=(kb == nkb - 1))
                    # softmax denominator: ones-row partition sum on PE
                    nc.tensor.matmul(ds_ps[0:1, offs[kb]:], ones_sb[:],
                                     pbs[kb][:, :nn],
                                     start=(kb == 0), stop=(kb == nkb - 1))

                for kb in range(nkb):
                    sub = kb - QB * qc
                    off = P * sub if sub >= 0 else 0
                    offs[kb] = off
                    nn = QCH - off
                    s_ps = psA.tile([P, QCH], f32, tag="A")
                    nc.tensor.matmul(s_ps[:, :nn], kT_sb[:, kb * P:(kb + 1) * P],
                                     qT_sb[h][:, qc * QCH + off:(qc + 1) * QCH],
                                     start=True, stop=True)
                    if sub >= 0:
                        nc.vector.tensor_tensor(s_ps[:, 0:P], s_ps[:, 0:P], mask_sb[:],
                                                mybir.AluOpType.add)
                    pb = ppool.tile([P, QCH], bf16, tag="pb")
                    nc.scalar.activation(pb[:, :nn], s_ps[:, :nn],
                                         mybir.ActivationFunctionType.Exp)
                    pbs[kb] = pb
                    if kb >= LOOKAHEAD:
                        emit_av(kb - LOOKAHEAD)
                for kb in range(max(0, nkb - LOOKAHEAD), nkb):
                    emit_av(kb)
                attn_state[h] = (y_ps, ds_ps)

            def attn_tail(h, qc):
                """Denominator -> reciprocal -> broadcast -> normalize."""
                y_ps, ds_ps = attn_state.pop(h)
                recip = work.tile([1, QCH], f32, tag="recip")
                nc.vector.reciprocal_approx_fast(recip[:], ds_ps[:])
                rbc = work.tile([P, QCH], f32, tag="rbc")
                nc.gpsimd.partition_broadcast(rbc[:], recip[:])
                yb = work.tile([P, QCH], bf16, tag=f"ybf{h}")
                nc.vector.tensor_tensor(yb[:], y_ps[:], rbc[:], mybir.AluOpType.mult)
                ybf[h] = yb

            def out_proj(qc):
                ybf_now = list(ybf)
                for t in range(QB):
                    stage = ost.tile([P, C], f32, tag="stage")
                    for n in range(C // 512):
                        o_ps = psC.tile([P, 512], f32, tag="C")
                        for h in range(NR):
                            nc.tensor.matmul(o_ps[:], ybf_now[h][:, t * P:(t + 1) * P],
                                             wo_sb[:, h, n * 512:(n + 1) * 512],
                                             start=(h == 0), stop=(h == NR - 1))
                        nc.vector.tensor_copy(stage[:, n * 512:(n + 1) * 512], o_ps[:])
                    r0 = (qc * QB + t) * P
                    nc.sync.dma_start(out_d[r0:r0 + P, 0:C // 2], stage[:, 0:C // 2])
                    nc.sync.dma_start(out_d[r0:r0 + P, C // 2:C], stage[:, C // 2:C])

            # Global emission schedule: software-pipelined so the PE stream
            # never waits on a just-emitted DVE/ACT chain. Proj fronts run one
            # token-block ahead of their tails (crossing q-chunk boundaries),
            # and out-proj of the previous q-chunk fills the tail chains.
            def attn_block(qc):
                attn_front(0, qc)
                attn_front(1, qc)
                attn_tail(0, qc)
                attn_front(2, qc)
                attn_tail(1, qc)
                attn_front(3, qc)
                attn_tail(2, qc)
                attn_tail(3, qc)

            proj_front(0)
            proj_front(1)
            proj_tail(0)
            proj_front(2)
            proj_tail(1)
            proj_front(3)
            proj_tail(2)
            proj_front(4)
            proj_tail(3)
            attn_block(0)
            for qc in range(1, NQC):
                b = qc * QB
                proj_front(b + 1)
                proj_tail(b)
                proj_front(b + 2)
                proj_tail(b + 1)
                proj_front(b + 3)
                proj_tail(b + 2)
                out_proj(qc - 1)
                if b + 4 < TB:
                    proj_front(b + 4)
                proj_tail(b + 3)
                attn_block(qc)
            out_proj(NQC - 1)

    nc.compile()
    return nc


def _host_prep(x, wq, wk, wv, wo, q_norm_w, k_norm_w):
    """Shard + lay out inputs for the 8 cores. Returns list of 8 in_maps."""
    inv_freq = 1.0 / (THETA ** (np.arange(0, HD, 2, dtype=np.float64) / HD))
    t = np.arange(T, dtype=np.float64)
    freqs = np.outer(t, inv_freq)
    emb = np.concatenate([freqs, freqs], axis=-1)
    cos = np.cos(emb)
    sin = np.sin(emb)

    def fold(w):
        rotw = np.concatenate([w[64:], w[:64]]).astype(np.float64)
        cosw = (cos * w[None, :].astype(np.float64)).astype(np.float32)
        sinw = (sin * rotw[None, :]).astype(np.float32)
        return (np.ascontiguousarray(cosw.reshape(TB, P, HD).transpose(1, 0, 2)),
                np.ascontiguousarray(sinw.reshape(TB, P, HD).transpose(1, 0, 2)))

    cosq, sinq = fold(q_norm_w)
    cosk, sink = fold(k_norm_w)

    # mask[s, t]: 0 if s <= t else -1e9 (scores^T diagonal-block causal mask)
    mask = np.where(np.arange(P)[:, None] <= np.arange(P)[None, :], 0.0,
                    MASK_NEG).astype(np.float32)
    ident = np.eye(P, dtype=BF16)
    ones = np.ones((P, 1), dtype=BF16)

    in_maps = []
    for b in range(B):
        # [T, C] -> tiles [TB, P(tok), KC, P(c)] with partition = c-inner
        xt = np.ascontiguousarray(
            x[b].reshape(TB, P, KC, P).transpose(0, 3, 2, 1)).astype(BF16)
        for g in range(N_KV):
            wqT = np.ascontiguousarray(
                wq[g * G_HD:(g + 1) * G_HD, :].T.reshape(KC, P, G_HD)
                .transpose(1, 0, 2)).astype(BF16)
            wkT = np.ascontiguousarray(
                wk[g * HD:(g + 1) * HD, :].T.reshape(KC, P, HD)
                .transpose(1, 0, 2)).astype(BF16)
            wvT = np.ascontiguousarray(
                wv[g * HD:(g + 1) * HD, :].T.reshape(KC, P, HD)
                .transpose(1, 0, 2)).astype(BF16)
            woT = np.ascontiguousarray(
                wo[:, g * G_HD:(g + 1) * G_HD].T.reshape(NR, P, C)
                .transpose(1, 0, 2)).astype(BF16)
            in_maps.append({
                "xt": xt, "wqT": wqT, "wkT": wkT, "wvT": wvT, "woT": woT,
                "cosq": cosq, "sinq": sinq, "cosk": cosk, "sink": sink,
                "mask": mask, "ident": ident, "ones": ones,
            })
    return in_maps


def kernel(x, wq, wk, wv, wo, q_norm_w, k_norm_w, _want_trace=False):
    x = np.asarray(x, dtype=np.float32)
    wq = np.asarray(wq, dtype=np.float32)
    wk = np.asarray(wk, dtype=np.float32)
    wv = np.asarray(wv, dtype=np.float32)
    wo = np.asarray(wo, dtype=np.float32)
    q_norm_w = np.asarray(q_norm_w, dtype=np.float32)
    k_norm_w = np.asarray(k_norm_w, dtype=np.float32)

    if "nc" not in _CACHED:
        _CACHED["nc"] = _build_nc()
    nc = _CACHED["nc"]

    in_maps = _host_prep(x, wq, wk, wv, wo, q_norm_w, k_norm_w)
    res = run_bass_kernel_spmd(nc, in_maps, list(range(8)), trace=_want_trace)
    if _want_trace:
        _CACHED["last_result"] = res

    out = np.zeros((B, T, C), dtype=np.float32)
    for b in range(B):
        for g in range(N_KV):
            out[b] += res.results[b * N_KV + g]["out"]
    return out


# revision 28
# speedup vs baseline: 1.0084x; 1.0084x over previous
"""Trainium2 Bass kernel for nn_CausalSelfAttention_52012053954857.

Full-input contract: kernel(**inputs) takes the unsharded fp32 inputs,
shards across 8 NeuronCores (tensor-parallel over the 4 KV-head groups x
data-parallel over batch 2), runs one SPMD Bass/Tile kernel, and gathers
(sum over TP ranks, stack over batch) the full [2, 2048, 2048] output.

Per-core dataflow (all matmuls bf16 with fp32 PSUM accumulation):
  x-tiles (stationary) @ wq/wk/wv -> q,k,v in [tokens, dims] layout
  RMSNorm via ACT square+rsqrt (norm weights + 1/sqrt(HD) folded into
  host-precomputed rope tables), RoPE on DVE, cast bf16
  PE-transpose q,k into [HD, T]; v stays [T, HD]
  scores^T[kpos, qpos] = kT_block.T @ qT_chunk (causal: restricted N)
  diagonal-block causal mask added in PSUM, exp on ACT -> probs bf16
  y^T[HD, qpos] accumulates v.T @ probs^T in PSUM over k-blocks
  softmax denominator: DVE-accumulated probs, ones-matmul partition sum,
  ACT reciprocal, gpsimd partition_broadcast, DVE multiply -> y bf16
  out rows = y^T.T @ woT, summed over the 4 heads, fp32 out.
"""
import os
import sys

for _p in ("/opt/trn_rl_repo", "/opt/pypackages"):
    if _p not in sys.path and os.path.isdir(_p):
        sys.path.append(_p)

import numpy as np
import ml_dtypes

import concourse.mybir as mybir
import concourse.tile as tile
from concourse import bacc
from concourse.bass_utils import run_bass_kernel_spmd

BF16 = ml_dtypes.bfloat16

B, T, C = 2, 2048, 2048
N_HEAD, N_KV, HD = 16, 4, 128
NR = N_HEAD // N_KV          # 4 q-heads per kv group
G_HD = NR * HD               # 512 q dims per group
THETA = 500000.0
EPS = 1e-6
P = 128
TB = T // P                  # 16 token blocks
KC = C // P                  # 16 contraction chunks
QCH = 512                    # q-chunk width for attention
NQC = T // QCH               # 4 q chunks
QB = QCH // P                # 4 token blocks per q chunk
MASK_NEG = -1e9

_CACHED = {}


def _build_nc():
    nc = bacc.Bacc("TRN2", target_bir_lowering=False, debug=False, num_devices=8)
    f32, bf16 = mybir.dt.float32, mybir.dt.bfloat16

    xt_d = nc.dram_tensor("xt", [TB, P, KC, P], bf16, kind="ExternalInput")
    wq_d = nc.dram_tensor("wqT", [P, KC, G_HD], bf16, kind="ExternalInput")
    wk_d = nc.dram_tensor("wkT", [P, KC, HD], bf16, kind="ExternalInput")
    wv_d = nc.dram_tensor("wvT", [P, KC, HD], bf16, kind="ExternalInput")
    wo_d = nc.dram_tensor("woT", [P, NR, C], bf16, kind="ExternalInput")
    cq_d = nc.dram_tensor("cosq", [P, TB, HD], f32, kind="ExternalInput")
    sq_d = nc.dram_tensor("sinq", [P, TB, HD], f32, kind="ExternalInput")
    ck_d = nc.dram_tensor("cosk", [P, TB, HD], f32, kind="ExternalInput")
    sk_d = nc.dram_tensor("sink", [P, TB, HD], f32, kind="ExternalInput")
    mask_d = nc.dram_tensor("mask", [P, P], f32, kind="ExternalInput")
    ident_d = nc.dram_tensor("ident", [P, P], bf16, kind="ExternalInput")
    ones_d = nc.dram_tensor("ones", [P, 1], bf16, kind="ExternalInput")
    out_d = nc.dram_tensor("out", [T, C], f32, kind="ExternalOutput")

    with tile.TileContext(nc) as tc:
        with (
            tc.tile_pool(name="const", bufs=1) as const,
            tc.tile_pool(name="xp", bufs=3) as xp,
            tc.tile_pool(name="work", bufs=2) as work,
            tc.tile_pool(name="probs", bufs=6) as ppool,
            tc.tile_pool(name="ost", bufs=2) as ost,
            tc.tile_pool(name="psA", bufs=4, space="PSUM") as psA,   # q-proj / scores
            tc.tile_pool(name="psB", bufs=2, space="PSUM") as psB,   # kv-proj / y
            tc.tile_pool(name="psC", bufs=2, space="PSUM") as psC,   # ds / out-proj
        ):
            # ---- persistent SBUF ----
            wq_sb = const.tile([P, KC, G_HD], bf16, tag="wq")
            wk_sb = const.tile([P, KC, HD], bf16, tag="wk")
            wv_sb = const.tile([P, KC, HD], bf16, tag="wv")
            wo_sb = const.tile([P, NR, C], bf16, tag="wo")
            cq_sb = const.tile([P, TB, HD], f32, tag="cq")
            sq_sb = const.tile([P, TB, HD], f32, tag="sq")
            ck_sb = const.tile([P, TB, HD], f32, tag="ck")
            sk_sb = const.tile([P, TB, HD], f32, tag="sk")
            mask_sb = const.tile([P, P], f32, tag="mask")
            ident_sb = const.tile([P, P], bf16, tag="ident")
            ones_sb = const.tile([P, 1], bf16, tag="ones")
            qT_sb = [const.tile([P, T], bf16, tag=f"qT{h}", name=f"qT{h}")
                     for h in range(NR)]
            kT_sb = const.tile([P, T], bf16, tag="kT")
            v_sb = const.tile([P, TB, HD], bf16, tag="v")

            # weights needed first, in parallel chunks; tables/wo later
            for c in range(0, KC, 4):
                nc.sync.dma_start(wq_sb[:, c:c + 4, :], wq_d[:, c:c + 4, :])
            nc.sync.dma_start(wk_sb[:], wk_d[:])
            nc.sync.dma_start(wv_sb[:], wv_d[:])
            nc.sync.dma_start(mask_sb[:], mask_d[:])
            nc.sync.dma_start(ident_sb[:], ident_d[:])
            nc.sync.dma_start(ones_sb[:], ones_d[:])
            nc.sync.dma_start(cq_sb[:], cq_d[:])
            nc.sync.dma_start(sq_sb[:], sq_d[:])
            nc.sync.dma_start(ck_sb[:], ck_d[:])
            nc.sync.dma_start(sk_sb[:], sk_d[:])
            for c in range(0, NR):
                nc.sync.dma_start(wo_sb[:, c, :], wo_d[:, c, :])

            epsq_sb = const.tile([P, 1], f32, tag="epsq")
            epsk_sb = const.tile([P, 1], f32, tag="epsk")
            nc.gpsimd.memset(epsq_sb[:], float(HD * EPS))
            nc.gpsimd.memset(epsk_sb[:], float(EPS))

            ybf = [None] * NR  # per-head normalized y^T [HD, QCH] bf16, per q-chunk
            proj_state = {}    # tb -> (q_ps, kv_ps)

            def proj_front(tb):
                """QKV projection matmuls for token block tb."""
                xt = xp.tile([P, KC, P], bf16, tag="xt")
                nc.sync.dma_start(xt[:], xt_d[tb])

                q_ps = psA.tile([P, G_HD], f32, tag="A")
                kv_ps = psB.tile([P, 2, HD], f32, tag="B")
                for kc in range(KC):
                    nc.tensor.matmul(q_ps[:], xt[:, kc], wq_sb[:, kc],
                                     start=(kc == 0), stop=(kc == KC - 1))
                for kc in range(KC):
                    nc.tensor.matmul(kv_ps[:, 0], xt[:, kc], wk_sb[:, kc],
                                     start=(kc == 0), stop=(kc == KC - 1))
                for kc in range(KC):
                    nc.tensor.matmul(kv_ps[:, 1], xt[:, kc], wv_sb[:, kc],
                                     start=(kc == 0), stop=(kc == KC - 1))
                proj_state[tb] = (q_ps, kv_ps)

            def proj_tail(tb):
                """Norm + rope + transposes for token block tb."""
                q_ps, kv_ps = proj_state.pop(tb)

                # v: straight cast copy into persistent [kpos, HD] store
                nc.scalar.activation(v_sb[:, tb], kv_ps[:, 1],
                                     mybir.ActivationFunctionType.Copy)

                # ---- RMSNorm stats (ACT square -> DVE segmented reduce) ----
                sq_scr = work.tile([P, NR + 1, HD], f32, tag="sq_scr")
                sums = work.tile([P, NR + 1], f32, tag="sums")
                rinv = work.tile([P, NR + 1], f32, tag="rinv")
                q3 = q_ps[:].rearrange("p (r d) -> p r d", r=NR)
                sq_flat = sq_scr[:].rearrange("p r d -> p (r d)")
                nc.scalar.activation(sq_flat[:, 0:G_HD], q_ps[:],
                                     mybir.ActivationFunctionType.Square)
                nc.scalar.activation(sq_flat[:, G_HD:G_HD + HD], kv_ps[:, 0],
                                     mybir.ActivationFunctionType.Square)
                nc.vector.reduce_sum(sums[:], sq_scr[:], axis=mybir.AxisListType.X)
                # q: rsqrt(sum + HD*eps) = rsqrt(mean+eps)/sqrt(HD)  (scores scale folded)
                # k: rsqrt(mean + eps); ACT Rsqrt is banned -> Sqrt + DVE reciprocal
                rt = work.tile([P, NR + 1], f32, tag="rt")
                nc.scalar.activation(rt[:, 0:NR], sums[:, 0:NR],
                                     mybir.ActivationFunctionType.Sqrt,
                                     bias=epsq_sb[:], scale=1.0)
                nc.scalar.activation(rt[:, NR:NR + 1], sums[:, NR:NR + 1],
                                     mybir.ActivationFunctionType.Sqrt,
                                     bias=epsk_sb[:], scale=1.0 / HD)
                nc.vector.reciprocal_approx_fast(rinv[:], rt[:])

                # ---- RoPE (DVE) ----
                # q
                rot = work.tile([P, NR, 2, 64], f32, tag="rot")
                q4 = q_ps[:].rearrange("p (r s d) -> p r s d", r=NR, s=2)
                nc.vector.tensor_scalar_mul(rot[:, :, 0, :], q4[:, :, 1, :], -1.0)
                nc.vector.tensor_copy(rot[:, :, 1, :], q4[:, :, 0, :])
                rot3 = rot[:].rearrange("p r s d -> p r (s d)")
                t1 = work.tile([P, NR, HD], f32, tag="t1")
                t2 = work.tile([P, NR, HD], f32, tag="t2")
                cq_b = cq_sb[:, tb:tb + 1, :].to_broadcast((P, NR, HD))
                sq_b = sq_sb[:, tb:tb + 1, :].to_broadcast((P, NR, HD))
                nc.vector.tensor_tensor(t1[:], q3, cq_b, mybir.AluOpType.mult)
                nc.vector.tensor_tensor(t2[:], rot3, sq_b, mybir.AluOpType.mult)
                qr = work.tile([P, NR, HD], f32, tag="qr")
                nc.vector.tensor_tensor(qr[:], t1[:], t2[:], mybir.AluOpType.add)
                qbf = work.tile([P, NR, HD], bf16, tag="qbf")
                rinvq_b = rinv[:, 0:NR, None].to_broadcast((P, NR, HD))
                nc.vector.tensor_tensor(qbf[:], qr[:], rinvq_b, mybir.AluOpType.mult)
                # k
                krot = work.tile([P, 2, 64], f32, tag="krot")
                k3 = kv_ps[:, 0].rearrange("p (s d) -> p s d", s=2)
                nc.vector.tensor_scalar_mul(krot[:, 0, :], k3[:, 1, :], -1.0)
                nc.vector.tensor_copy(krot[:, 1, :], k3[:, 0, :])
                krot2 = krot[:].rearrange("p s d -> p (s d)")
                kt1 = work.tile([P, HD], f32, tag="kt1")
                kt2 = work.tile([P, HD], f32, tag="kt2")
                nc.vector.tensor_tensor(kt1[:], kv_ps[:, 0], ck_sb[:, tb], mybir.AluOpType.mult)
                nc.vector.tensor_tensor(kt2[:], krot2, sk_sb[:, tb], mybir.AluOpType.mult)
                kr = work.tile([P, HD], f32, tag="kr")
                nc.vector.tensor_tensor(kr[:], kt1[:], kt2[:], mybir.AluOpType.add)
                kbf = work.tile([P, HD], bf16, tag="kbf")
                rinvk_b = rinv[:, NR:NR + 1].to_broadcast((P, HD))
                nc.vector.tensor_tensor(kbf[:], kr[:], rinvk_b, mybir.AluOpType.mult)

                # ---- transposes to [HD, T] (PE) ----
                qbf2 = qbf[:].rearrange("p r d -> p (r d)")
                for h in range(NR):
                    tp = psC.tile([P, P], bf16, tag="C")
                    nc.tensor.transpose(tp[:], qbf2[:, h * HD:(h + 1) * HD], ident_sb[:])
                    nc.scalar.activation(qT_sb[h][:, tb * P:(tb + 1) * P], tp[:],
                                         mybir.ActivationFunctionType.Copy)
                tp = psC.tile([P, P], bf16, tag="C")
                nc.tensor.transpose(tp[:], kbf[:], ident_sb[:])
                nc.scalar.activation(kT_sb[:, tb * P:(tb + 1) * P], tp[:],
                                     mybir.ActivationFunctionType.Copy)

            attn_state = {}  # h -> (y_ps, ds_ps)
            LOOKAHEAD = 3

            def attn_front(h, qc):
                """QK + exp + AV for head h, q-chunk qc (AV trails QK by
                LOOKAHEAD so exp latency hides under PE work)."""
                nkb = QB * qc + QB
                y_ps = psB.tile([P, QCH], f32, tag="B")
                ds_ps = psC.tile([1, QCH], f32, tag="C")
                pbs = [None] * nkb
                offs = [0] * nkb

                def emit_av(kb):
                    nn = QCH - offs[kb]
                    nc.tensor.matmul(y_ps[:, offs[kb]:], v_sb[:, kb],
                                     pbs[kb][:, :nn],
                                     start=(kb == 0), stop=(kb == nkb - 1))
                    # softmax denominator: ones-row partition sum on PE
                    nc.tensor.matmul(ds_ps[0:1, offs[kb]:], ones_sb[:],
                                     pbs[kb][:, :nn],
                                     start=(kb == 0), stop=(kb == nkb - 1))

                for kb in range(nkb):
                    sub = kb - QB * qc
                    off = P * sub if sub >= 0 else 0
                    offs[kb] = off
                    nn = QCH - off
                    s_ps = psA.tile([P, QCH], f32, tag="A")
                    nc.tensor.matmul(s_ps[:, :nn], kT_sb[:, kb * P:(kb + 1) * P],
                                     qT_sb[h][:, qc * QCH + off:(qc + 1) * QCH],
                                     start=True, stop=True)
                    if sub >= 0:
                        nc.vector.tensor_tensor(s_ps[:, 0:P], s_ps[:, 0:P], mask_sb[:],
                                                mybir.AluOpType.add)
                    pb = ppool.tile([P, QCH], bf16, tag="pb")
                    nc.scalar.activation(pb[:, :nn], s_ps[:, :nn],
                                         mybir.ActivationFunctionType.Exp)
                    pbs[kb] = pb
                    if kb >= LOOKAHEAD:
                        emit_av(kb - LOOKAHEAD)
                for kb in range(max(0, nkb - LOOKAHEAD), nkb):
                    emit_av(kb)
                attn_state[h] = (y_ps, ds_ps)

            def attn_tail(h, qc):
                """Denominator -> reciprocal -> broadcast -> normalize."""
                y_ps, ds_ps = attn_state.pop(h)
                recip = work.tile([1, QCH], f32, tag="recip")
                nc.vector.reciprocal_approx_fast(recip[:], ds_ps[:])
                rbc = work.tile([P, QCH], f32, tag="rbc")
                nc.gpsimd.partition_broadcast(rbc[:], recip[:])
                yb = work.tile([P, QCH], bf16, tag=f"ybf{h}")
                nc.vector.tensor_tensor(yb[:], y_ps[:], rbc[:], mybir.AluOpType.mult)
                ybf[h] = yb

            def out_proj(qc):
                ybf_now = list(ybf)
                for t in range(QB):
                    stage = ost.tile([P, C], f32, tag="stage")
                    for n in range(C // 512):
                        o_ps = psC.tile([P, 512], f32, tag="C")
                        for h in range(NR):
                            nc.tensor.matmul(o_ps[:], ybf_now[h][:, t * P:(t + 1) * P],
                                             wo_sb[:, h, n * 512:(n + 1) * 512],
                                             start=(h == 0), stop=(h == NR - 1))
                        nc.vector.tensor_copy(stage[:, n * 512:(n + 1) * 512], o_ps[:])
                        r0 = (qc * QB + t) * P
                        nc.sync.dma_start(out_d[r0:r0 + P, n * 512:(n + 1) * 512],
                                          stage[:, n * 512:(n + 1) * 512])

            # Global emission schedule: software-pipelined so the PE stream
            # never waits on a just-emitted DVE/ACT chain. Proj fronts run one
            # token-block ahead of their tails (crossing q-chunk boundaries),
            # and out-proj of the previous q-chunk fills the tail chains.
            def attn_block(qc):
                attn_front(0, qc)
                attn_front(1, qc)
                attn_tail(0, qc)
                attn_front(2, qc)
                attn_tail(1, qc)
                attn_front(3, qc)
                attn_tail(2, qc)
                attn_tail(3, qc)

            proj_front(0)
            proj_front(1)
            proj_tail(0)
            proj_front(2)
            proj_tail(1)
            proj_front(3)
            proj_tail(2)
            proj_front(4)
            proj_tail(3)
            attn_block(0)
            for qc in range(1, NQC):
                b = qc * QB
                proj_front(b + 1)
                proj_tail(b)
                proj_front(b + 2)
                proj_tail(b + 1)
                proj_front(b + 3)
                proj_tail(b + 2)
                out_proj(qc - 1)
                if b + 4 < TB:
                    proj_front(b + 4)
                proj_tail(b + 3)
                attn_block(qc)
            out_proj(NQC - 1)

    nc.compile()
    return nc


def _host_prep(x, wq, wk, wv, wo, q_norm_w, k_norm_w):
    """Shard + lay out inputs for the 8 cores. Returns list of 8 in_maps."""
    inv_freq = 1.0 / (THETA ** (np.arange(0, HD, 2, dtype=np.float64) / HD))
    t = np.arange(T, dtype=np.float64)
    freqs = np.outer(t, inv_freq)
    emb = np.concatenate([freqs, freqs], axis=-1)
    cos = np.cos(emb)
    sin = np.sin(emb)

    def fold(w):
        rotw = np.concatenate([w[64:], w[:64]]).astype(np.float64)
        cosw = (cos * w[None, :].astype(np.float64)).astype(np.float32)
        sinw = (sin * rotw[None, :]).astype(np.float32)
        return (np.ascontiguousarray(cosw.reshape(TB, P, HD).transpose(1, 0, 2)),
                np.ascontiguousarray(sinw.reshape(TB, P, HD).transpose(1, 0, 2)))

    cosq, sinq = fold(q_norm_w)
    cosk, sink = fold(k_norm_w)

    # mask[s, t]: 0 if s <= t else -1e9 (scores^T diagonal-block causal mask)
    mask = np.where(np.arange(P)[:, None] <= np.arange(P)[None, :], 0.0,
                    MASK_NEG).astype(np.float32)
    ident = np.eye(P, dtype=BF16)
    ones = np.ones((P, 1), dtype=BF16)

    in_maps = []
    for b in range(B):
        # [T, C] -> tiles [TB, P(tok), KC, P(c)] with partition = c-inner
        xt = np.ascontiguousarray(
            x[b].reshape(TB, P, KC, P).transpose(0, 3, 2, 1)).astype(BF16)
        for g in range(N_KV):
            wqT = np.ascontiguousarray(
                wq[g * G_HD:(g + 1) * G_HD, :].T.reshape(KC, P, G_HD)
                .transpose(1, 0, 2)).astype(BF16)
            wkT = np.ascontiguousarray(
                wk[g * HD:(g + 1) * HD, :].T.reshape(KC, P, HD)
                .transpose(1, 0, 2)).astype(BF16)
            wvT = np.ascontiguousarray(
                wv[g * HD:(g + 1) * HD, :].T.reshape(KC, P, HD)
                .transpose(1, 0, 2)).astype(BF16)
            woT = np.ascontiguousarray(
                wo[:, g * G_HD:(g + 1) * G_HD].T.reshape(NR, P, C)
                .transpose(1, 0, 2)).astype(BF16)
            in_maps.append({
                "xt": xt, "wqT": wqT, "wkT": wkT, "wvT": wvT, "woT": woT,
                "cosq": cosq, "sinq": sinq, "cosk": cosk, "sink": sink,
                "mask": mask, "ident": ident, "ones": ones,
            })
    return in_maps


def kernel(x, wq, wk, wv, wo, q_norm_w, k_norm_w, _want_trace=False):
    x = np.asarray(x, dtype=np.float32)
    wq = np.asarray(wq, dtype=np.float32)
    wk = np.asarray(wk, dtype=np.float32)
    wv = np.asarray(wv, dtype=np.float32)
    wo = np.asarray(wo, dtype=np.float32)
    q_norm_w = np.asarray(q_norm_w, dtype=np.float32)
    k_norm_w = np.asarray(k_norm_w, dtype=np.float32)

    if "nc" not in _CACHED:
        _CACHED["nc"] = _build_nc()
    nc = _CACHED["nc"]

    in_maps = _host_prep(x, wq, wk, wv, wo, q_norm_w, k_norm_w)
    res = run_bass_kernel_spmd(nc, in_maps, list(range(8)), trace=_want_trace)
    if _want_trace:
        _CACHED["last_result"] = res

    out = np.zeros((B, T, C), dtype=np.float32)
    for b in range(B):
        for g in range(N_KV):
            out[b] += res.results[b * N_KV + g]["out"]
    return out
